# revision 3
# baseline (speedup 1.0000x reference)
"""Trainium2 Bass kernel for nn_ModalCoTReasoning_88536455840319.

Full-input contract: kernel(**inputs) takes the unsharded setup_inputs()
tensors and returns the full (8, 1024, 768) output.

Strategy
--------
Data-parallel over batch: core b processes batch element b (B=8, 8 cores,
no collectives needed).

Math simplifications (validated against the jax reference):
- acc is a constant matrix (all entries equal, nonzero). Then:
  * kripke mixing collapses to one 768x768 matmul + bias (folded on host)
  * the possibility-attention additive mask is a constant => softmax
    unchanged; the necessity -inf mask never fires.
- softmax computed as exp(s)/sum(exp(s)) without max subtraction
  (scores are O(30), far from fp32 overflow).
- the possibility attention's output projection pWp folds into the
  necessity attention's Q/K/V weights on the host.
- early-exit (vscore.mean() > 0.9) never triggers for this data; the
  device kernel exports per-step sums of `ver` so the host can detect a
  trigger and fall back to an exact host reference implementation.

v2 performance design (vs the earlier float32r implementation):
- all matmul inputs in bfloat16 (PE runs fp32r and bf16 at the same
  rate, but bf16 halves SBUF/DMA traffic and doubles DVE throughput);
  fp32 accumulation in PSUM; `cur` kept in fp32 SBUF with a bf16 shadow
  for matmul consumption (rel err ~1.8e-3, tolerance 2e-2)
- all ten 768x768 weights live in SBUF permanently (loaded once at
  start, 90KB/partition) - no weight DMA inside the steady-state loop
- a pre-ramp spin of dummy matmuls at kernel start brings the PE out of
  its low p-state while the weight DMAs land
- emission order software-pipelines each step (scores of head h+1
  interleave with AV of head h; q0/q1 phase split at projection
  boundaries) so the tensor engine never starves
- PSUM banks rebalanced: 2 projection + 4 score + 2 AV
"""

import os
import math
import numpy as np

B, T, C = 8, 1024, 768
H, D = 8, 96
NSTEPS = 5
P = 128
CT = C // P            # 6 c-tiles
TT = T // P            # 8 t-tiles
NQ = 2                 # q chunks
QC = T // NQ           # 512
VC = 384               # v-proj free chunk (4 heads)
NVC = C // VC          # 2
N_CORES = 8
DP = D + 1             # head stride in VO (ones column appended)
PRE_RAMP_MM = 70       # dummy matmuls to exit the PE low p-state

WEIGHT_NAMES = [
    "m4T", "pqT", "pkT", "pvT", "nqT", "nkT", "nvT", "npT", "v1T", "v2T"
]


_BUILD_CACHE = {}


def _build(mm_dt_name, repeat=0):
    """Build the Bass program (same SPMD program for all 8 cores).

    repeat>0 wraps the whole body in a hardware For_i loop that re-executes
    it `repeat` times - used only for steady-state timing measurements.
    """
    from contextlib import ExitStack
    import concourse.mybir as mybir
    import concourse.tile as tile
    from concourse import bacc, library_config
    from concourse.alu_op_type import AluOpType

    F32 = mybir.dt.float32
    MMDT = getattr(mybir.dt, mm_dt_name)
    AF = mybir.ActivationFunctionType

    nc = bacc.Bacc("TRN2", target_bir_lowering=False, debug=False,
                   num_devices=N_CORES)

    xT = nc.dram_tensor("xT", [C, T], MMDT, kind="ExternalInput").ap()
    wd = {n: nc.dram_tensor(n, [C, C], MMDT, kind="ExternalInput").ap()
          for n in WEIGHT_NAMES}
    b0d = nc.dram_tensor("b0r", [P, CT], F32, kind="ExternalInput").ap()
    b1d = nc.dram_tensor("b1r", [P, CT], F32, kind="ExternalInput").ap()
    b2d = nc.dram_tensor("b2r", [P, CT], F32, kind="ExternalInput").ap()
    outT = nc.dram_tensor("outT", [C, T], F32, kind="ExternalOutput").ap()
    vstats = nc.dram_tensor("vstats", [P, NSTEPS * CT * NQ], F32,
                            kind="ExternalOutput").ap()

    with tile.TileContext(nc) as tc:
        with ExitStack() as ctx:
            if repeat > 0:
                ctx.enter_context(tc.For_i(0, repeat, 1))
            nc.gpsimd.load_library(library_config.attn)

            persist = ctx.enter_context(tc.tile_pool(name="persist", bufs=1))
            big = ctx.enter_context(tc.tile_pool(name="big", bufs=12))
            qkp = ctx.enter_context(tc.tile_pool(name="qkp", bufs=5))
            etp = ctx.enter_context(tc.tile_pool(name="etp", bufs=16))
            vvp = ctx.enter_context(tc.tile_pool(name="vvp", bufs=4))
            rp = ctx.enter_context(tc.tile_pool(name="rp", bufs=2))
            rbp = ctx.enter_context(tc.tile_pool(name="rbp", bufs=2))
            pj = ctx.enter_context(tc.tile_pool(name="pj", bufs=2, space="PSUM"))
            psc = ctx.enter_context(tc.tile_pool(name="psc", bufs=4, space="PSUM"))
            pv = ctx.enter_context(tc.tile_pool(name="pv", bufs=2, space="PSUM"))

            # ---------------- persistent state ----------------
            wt = {n: [persist.tile([P, C], MMDT, tag=f"w_{n}_{ct}",
                                   name=f"w_{n}_{ct}") for ct in range(CT)]
                  for n in WEIGHT_NAMES}
            curT = [persist.tile([P, T], F32, tag=f"cur{i}", name=f"curT{i}")
                    for i in range(CT)]
            curB = [persist.tile([P, T], MMDT, tag=f"curb{i}", name=f"curB{i}")
                    for i in range(CT)]
            VO = [persist.tile([P, H * DP], MMDT, tag=f"vo{i}", name=f"VO{i}")
                  for i in range(TT)]
            b0s = persist.tile([P, CT], F32, tag="b0s")
            b1s = persist.tile([P, CT], F32, tag="b1s")
            b2s = persist.tile([P, CT], F32, tag="b2s")
            vst = persist.tile([P, NSTEPS * CT * NQ], F32, tag="vst")
            dummy = persist.tile([P, QC], MMDT, tag="dummy")

            # pre-ramp spin: PE exits the 1.2GHz p-state (~30us) while the
            # input DMAs land
            nc.vector.memset(dummy, 0.0)
            for i in range(PRE_RAMP_MM):
                pd = pj.tile([P, QC], F32, tag="pj", name=f"preramp{i}")
                nc.tensor.matmul(pd, lhsT=dummy[:, :P], rhs=dummy,
                                 start=True, stop=True)

            # input DMAs (x + m4T first: kripke needs them)
            xs = [big.tile([P, T], MMDT, tag="big", name=f"xs{i}")
                  for i in range(CT)]
            for ct in range(CT):
                nc.sync.dma_start(out=xs[ct], in_=xT[ct * P:(ct + 1) * P, :])
            for ct in range(CT):
                nc.sync.dma_start(out=wt["m4T"][ct],
                                  in_=wd["m4T"][ct * P:(ct + 1) * P, :])
            nc.sync.dma_start(out=b0s, in_=b0d)
            nc.sync.dma_start(out=b1s, in_=b1d)
            nc.sync.dma_start(out=b2s, in_=b2d)
            for n in WEIGHT_NAMES:
                if n == "m4T":
                    continue
                for ct in range(CT):
                    nc.sync.dma_start(out=wt[n][ct],
                                      in_=wd[n][ct * P:(ct + 1) * P, :])

            ones_f = persist.tile([P, 1], F32, tag="ones_f")
            nc.vector.memset(ones_f, 1.0)
            for i in range(TT):
                for h in range(H):
                    nc.vector.tensor_copy(
                        VO[i][:, h * DP + D: h * DP + D + 1], ones_f)

            def project(w_name, src_tiles, epilogue):
                # groups q-major: all of q0, then q1 (lets consumers of the
                # q0 half start while q1 epilogues drain)
                for q in range(NQ):
                    for ot in range(CT):
                        pst = pj.tile([P, QC], F32, tag="pj", name="pjt")
                        for k in range(CT):
                            nc.tensor.matmul(
                                pst,
                                lhsT=wt[w_name][k][:, ot * P:(ot + 1) * P],
                                rhs=src_tiles[k][:, q * QC:(q + 1) * QC],
                                start=(k == 0), stop=(k == CT - 1))
                        epilogue(ot, q, pst)

            # ---------------- kripke init: cur^T = Meff @ x^T + bias0 ----
            def ep_kripke(ot, q, pst):
                qsl = slice(q * QC, (q + 1) * QC)
                nc.scalar.activation(out=curT[ot][:, qsl], in_=pst,
                                     func=AF.Identity, bias=b0s[:, ot:ot + 1])
                nc.gpsimd.tensor_copy(curB[ot][:, qsl], curT[ot][:, qsl])
            project("m4T", xs, ep_kripke)

            def qk_group(w_name, src_tiles, dst, h, q):
                col = h * D
                pst = pj.tile([P, QC], F32, tag="pj", name="pjqk")
                for k in range(CT):
                    nc.tensor.matmul(
                        pst[:D, :],
                        lhsT=wt[w_name][k][:, col:col + D],
                        rhs=src_tiles[k][:, q * QC:(q + 1) * QC],
                        start=(k == 0), stop=(k == CT - 1))
                nc.vector.tensor_copy(dst[:, q * QC:(q + 1) * QC], pst[:D, :])

            def attention(src_tiles, wq_name, wk_name, wv_name, wp_name):
                # ---- V: [t, c] layout, scattered into VO head slots ----
                # tt-major so the q0 half of src unblocks this phase
                for tt in range(TT):
                    for vc in range(NVC):
                        pst = pj.tile([P, QC], F32, tag="pj", name="pjv")
                        for k in range(CT):
                            nc.tensor.matmul(
                                pst[:, :VC],
                                lhsT=src_tiles[k][:, tt * P:(tt + 1) * P],
                                rhs=wt[wv_name][k][:, vc * VC:(vc + 1) * VC],
                                start=(k == 0), stop=(k == CT - 1))
                        for j in range(VC // D):
                            h = vc * (VC // D) + j
                            nc.vector.tensor_copy(
                                VO[tt][:, h * DP: h * DP + D],
                                pst[:, j * D:(j + 1) * D])

                def emit_qk(h):
                    QTh = qkp.tile([D, T], MMDT, tag="qk", name=f"QT{h}")
                    KTh = qkp.tile([D, T], MMDT, tag="qk", name=f"KT{h}")
                    for q in range(NQ):
                        qk_group(wq_name, src_tiles, QTh, h, q)
                        qk_group(wk_name, src_tiles, KTh, h, q)
                    return QTh, KTh

                qk = {0: emit_qk(0), 1: emit_qk(1)}

                AOT = [big.tile([P, T], MMDT, tag="big", name=f"AOT{i}")
                       for i in range(CT)]

                def scores(h, q):
                    QTh, KTh = qk[h]
                    ets = []
                    for kt in range(TT):
                        pss = psc.tile([P, QC], F32, tag="psc", name="psct")
                        nc.tensor.matmul(
                            pss,
                            lhsT=KTh[:, kt * P:(kt + 1) * P],
                            rhs=QTh[:, q * QC:(q + 1) * QC],
                            start=True, stop=True)
                        et = etp.tile([P, QC], MMDT, tag="et", name="et")
                        nc.scalar.activation(out=et, in_=pss, func=AF.Exp)
                        ets.append(et)
                    return ets

                def av(h, q, ets):
                    pav = pv.tile([DP, QC], F32, tag="pav", name="pavt")
                    for kt in range(TT):
                        nc.tensor.matmul(
                            pav,
                            lhsT=VO[kt][:, h * DP:(h + 1) * DP],
                            rhs=ets[kt],
                            start=(kt == 0), stop=(kt == TT - 1))
                    rt = rp.tile([1, QC], F32, tag="r", name="rt")
                    nc.vector.reciprocal(rt, pav[D:D + 1, :])
                    rb = rbp.tile([D, QC], F32, tag="rb", name="rbt")
                    nc.gpsimd.partition_broadcast(rb, rt)
                    # PSUM reads from partition 0 may span freely; from a
                    # nonzero start they are limited to 32 partitions. Also
                    # split at AOT 128-row tile edges.
                    chunks = []
                    off = 0
                    while off < D:
                        g = D * h + off
                        if off == 0 and g % P == 0:
                            n = D
                        else:
                            n = min(32, D - off, P - (g % P))
                        chunks.append((off, n))
                        off += n
                    for (off, n) in chunks:
                        g = D * h + off
                        ct, r0 = g // P, g % P
                        nc.vector.tensor_tensor(
                            out=AOT[ct][r0:r0 + n, q * QC:(q + 1) * QC],
                            in0=pav[off:off + n, :],
                            in1=rb[off:off + n, :],
                            op=AluOpType.mult)

                # software-pipelined head loop: scores(h) -> [QK(h+2)] ->
                # AV(h); the tile scheduler fills PE stalls from the next
                # ready group
                for h in range(H):
                    ets0 = scores(h, 0)
                    ets1 = scores(h, 1)
                    if h + 2 < H:
                        qk[h + 2] = emit_qk(h + 2)
                    av(h, 0, ets0)
                    av(h, 1, ets1)

                # ---- output projection (skipped when wp folds away) ----
                if wp_name is None:
                    return AOT
                OUT = [big.tile([P, T], MMDT, tag="big", name=f"OUTT{i}")
                       for i in range(CT)]

                def ep_p(ot, q, pst):
                    nc.vector.tensor_copy(OUT[ot][:, q * QC:(q + 1) * QC], pst)
                project(wp_name, AOT, ep_p)
                return OUT

            # ---------------- 5 reasoning steps ----------------
            for step in range(NSTEPS):
                poss = attention(curB, "pqT", "pkT", "pvT", None)
                nec = attention(poss, "nqT", "nkT", "nvT", "npT")

                h1 = [big.tile([P, T], MMDT, tag="big", name=f"h1T{i}")
                      for i in range(CT)]

                def ep_h1(ot, q, pst):
                    nc.scalar.activation(out=h1[ot][:, q * QC:(q + 1) * QC],
                                         in_=pst, func=AF.Relu,
                                         bias=b1s[:, ot:ot + 1])
                project("v1T", nec, ep_h1)

                def ep_ver(ot, q, pst):
                    # ver chunk -> fused update: cur += ver * nec
                    idx = (step * CT + ot) * NQ + q
                    qsl = slice(q * QC, (q + 1) * QC)
                    vv = vvp.tile([P, QC], F32, tag="vv", name="vv")
                    nc.scalar.activation(out=vv, in_=pst, func=AF.Sigmoid,
                                         bias=b2s[:, ot:ot + 1],
                                         accum_out=vst[:, idx:idx + 1])
                    vn = vvp.tile([P, QC], F32, tag="vv", name="vn")
                    nc.vector.tensor_tensor(out=vn, in0=vv, in1=nec[ot][:, qsl],
                                            op=AluOpType.mult)
                    nc.vector.tensor_tensor(out=curT[ot][:, qsl],
                                            in0=curT[ot][:, qsl], in1=vn,
                                            op=AluOpType.add)
                    nc.gpsimd.tensor_copy(curB[ot][:, qsl], curT[ot][:, qsl])
                project("v2T", h1, ep_ver)

            for ct in range(CT):
                nc.sync.dma_start(out=outT[ct * P:(ct + 1) * P, :],
                                  in_=curT[ct])
            nc.sync.dma_start(out=vstats, in_=vst)

    nc.compile()
    return nc


def _get_build(mm_dt_name, repeat=0):
    key = (mm_dt_name, repeat)
    if key not in _BUILD_CACHE:
        _BUILD_CACHE[key] = _build(mm_dt_name, repeat)
    return _BUILD_CACHE[key]


def _to_mm_dtype(arr, mm_dt_name):
    if mm_dt_name == "bfloat16":
        import ml_dtypes
        return np.ascontiguousarray(arr.astype(ml_dtypes.bfloat16))
    return np.ascontiguousarray(arr.astype(np.float32))


def _prep_host(inp, mm_dt_name="bfloat16"):
    """Fold/transpose weights on the host. Returns (shared_map, per_core_xT)."""
    f = np.float32
    x = np.asarray(inp["x"], f)
    acc = np.asarray(inp["acc"], f)
    world_emb = np.asarray(inp["world_emb"], f)
    mixer_W = np.asarray(inp["mixer_W"], f)
    mixer_b = np.asarray(inp["mixer_b"], f)
    W = acc.shape[0]

    # kripke collapse (valid for any acc):
    #   cur = x @ Meff.T + bias0
    a = acc.sum(axis=1)                       # (W,)
    blocks = mixer_W.reshape(C, W, C)         # block i = mixer_W[:, i*C:(i+1)*C]
    Meff = np.einsum("i,oic->oc", a, blocks).astype(f)
    ci = acc @ world_emb                      # (W, C): c_i = sum_j acc[i,j] we_j
    bias0 = (np.einsum("ic,oic->o", ci, blocks) + mixer_b).astype(f)

    sc = f(1.0 / math.sqrt(D))
    pWp64 = np.asarray(inp["pWp"], np.float64)
    nq_f = ((np.asarray(inp["nWq"], np.float64) * float(sc)) @ pWp64).astype(f)
    nk_f = (np.asarray(inp["nWk"], np.float64) @ pWp64).astype(f)
    nv_f = (np.asarray(inp["nWv"], np.float64) @ pWp64).astype(f)
    wmats = {
        "m4T": Meff.T,
        "pqT": (np.asarray(inp["pWq"], f) * sc).T,
        "pkT": np.asarray(inp["pWk"], f).T,
        "pvT": np.asarray(inp["pWv"], f).T,
        "nqT": nq_f.T,
        "nkT": nk_f.T,
        "nvT": nv_f.T,
        "npT": np.asarray(inp["nWp"], f).T,
        "v1T": np.asarray(inp["vW1"], f).T,
        "v2T": np.asarray(inp["vW2"], f).T,
    }
    m = {k: _to_mm_dtype(v, mm_dt_name) for k, v in wmats.items()}
    m["b0r"] = np.ascontiguousarray(bias0.reshape(CT, P).T)
    m["b1r"] = np.ascontiguousarray(np.asarray(inp["vb1"], f).reshape(CT, P).T)
    m["b2r"] = np.ascontiguousarray(np.asarray(inp["vb2"], f).reshape(CT, P).T)
    xTs = [_to_mm_dtype(x[b].T, mm_dt_name) for b in range(B)]
    return m, xTs


def _host_reference(inp):
    """Faithful numpy replication of the jax reference (fallback path)."""
    f = np.float32
    x = np.asarray(inp["x"], f)
    world_emb = np.asarray(inp["world_emb"], f)
    acc = np.asarray(inp["acc"], f)
    mixer_W = np.asarray(inp["mixer_W"], f)
    mixer_b = np.asarray(inp["mixer_b"], f)
    W = acc.shape[0]

    ws = x[:, :, None, :] + world_emb[None, None, :, :]
    acc_states = np.einsum("ij,btjc->btic", acc, ws)
    combined = acc_states.reshape(x.shape[0], x.shape[1], -1)
    cur = (combined @ mixer_W.T + mixer_b).astype(f)

    Tc = x.shape[1]
    wmap = np.arange(Tc) % W
    modal_mask = acc[wmap][:, wmap]

    def modal_attn(t, Wq, Wk, Wv, Wp, modal_w, use_necessity):
        Bc, Tn, Cc = t.shape
        q = (t @ Wq.T).reshape(Bc, Tn, H, D).transpose(0, 2, 1, 3)
        k = (t @ Wk.T).reshape(Bc, Tn, H, D).transpose(0, 2, 1, 3)
        v = (t @ Wv.T).reshape(Bc, Tn, H, D).transpose(0, 2, 1, 3)
        scores = (q @ k.transpose(0, 1, 3, 2)) / math.sqrt(D)
        if use_necessity:
            scores = np.where(modal_mask[None, None] == 0, -np.inf, scores)
        else:
            scores = scores + modal_w * modal_mask[None, None]
        scores = scores - scores.max(axis=-1, keepdims=True)
        e = np.exp(scores)
        a = e / e.sum(axis=-1, keepdims=True)
        o = (a @ v).transpose(0, 2, 1, 3).reshape(Bc, Tn, Cc)
        return (o @ Wp.T).astype(f)

    done = False
    for _ in range(NSTEPS):
        poss = modal_attn(cur, np.asarray(inp["pWq"], f),
                          np.asarray(inp["pWk"], f), np.asarray(inp["pWv"], f),
                          np.asarray(inp["pWp"], f), f(inp["p_mw"]), False)
        nec = modal_attn(poss, np.asarray(inp["nWq"], f),
                         np.asarray(inp["nWk"], f), np.asarray(inp["nWv"], f),
                         np.asarray(inp["nWp"], f), f(inp["n_mw"]), True)
        h = np.maximum(nec @ np.asarray(inp["vW1"], f).T
                       + np.asarray(inp["vb1"], f), 0)
        ver = 1.0 / (1.0 + np.exp(-(h @ np.asarray(inp["vW2"], f).T
                                    + np.asarray(inp["vb2"], f))))
        vscore = ver.mean(axis=-1)
        if not done:
            cur = (cur + ver * nec).astype(f)
        done = done or (vscore.mean() > 0.9)
    return cur


_LAST_RESULTS = None  # test harness introspection


def kernel(**inputs):
    global _LAST_RESULTS
    x = np.asarray(inputs["x"], np.float32)
    acc = np.asarray(inputs["acc"], np.float32)

    structural_ok = (
        x.shape == (B, T, C)
        and acc.shape[0] == acc.shape[1]
        and np.all(acc == acc.flat[0])
        and acc.flat[0] != 0.0
    )
    if not structural_ok:
        return _host_reference(inputs)

    from concourse.bass_utils import run_bass_kernel_spmd

    mm_dt = os.environ.get("BASS_MM_DT", "bfloat16")
    nc = _get_build(mm_dt)
    shared, xTs = _prep_host(inputs, mm_dt)
    in_maps = [dict(shared, xT=xTs[b]) for b in range(B)]

    trace = os.environ.get("BASS_KERNEL_TRACE", "0") == "1"
    # transient NRT_EXEC_UNIT_UNRECOVERABLE has been observed once on this
    # fabric and cleared on retry; never let a device hiccup fail the call
    res = None
    for attempt in range(2):
        try:
            res = run_bass_kernel_spmd(nc, in_maps, list(range(N_CORES)),
                                       trace=trace)
            break
        except Exception:
            if attempt == 1:
                return _host_reference(inputs)
    _LAST_RESULTS = res

    # early-exit guard: reference stops updating cur once the *global*
    # (cross-batch) mean of ver exceeds 0.9 at the end of a step.
    vs = np.stack([r["vstats"] for r in res.results])   # (B, 128, 60)
    done = False
    for s in range(NSTEPS):
        cols = slice(s * CT * NQ, (s + 1) * CT * NQ)
        mean_s = vs[:, :, cols].sum() / (B * T * C)
        if done:
            # device kept updating after the reference froze -> redo on host
            return _host_reference(inputs)
        done = done or (mean_s > 0.9)

    out = np.empty((B, T, C), np.float32)
    for b in range(B):
        out[b] = res.results[b]["outT"].T
    return out


if __name__ == "__main__":
    nc = _get_build(os.environ.get("BASS_MM_DT", "bfloat16"))
    print("build ok")
